# revision 32
# baseline (speedup 1.0000x reference)
"""Trainium2 Bass kernel for nn_DotProductAttention_61529701482813.

Reference computation (per batch b):
    Q = query @ Wq + bq ; K = key @ Wk + bk ; V = value @ Wv + bv
    S = Q @ K^T / sqrt(768)
    S = where(keep, S / 0.8, 0)        # dropout on raw scores, keep ~ Bern(0.8), jax key(42)
    att = softmax(S, axis=-1)
    att = where(mask == 0, 1e-9, att)  # mask applied AFTER softmax
    out = att @ V

Sharding: pure data parallel, batch b -> core b (B == n_cores == 8). No collectives.

Per-core kernel layout strategy:
  * All matmul operands are fp16 (PE runs 1 cycle/row for fp16 vs 4 for fp32);
    accumulation is always fp32 in PSUM.
  * Projections consume host-transposed activations xT [D, S] so the contraction
    dim (d) lands on partitions. Q and K are produced TRANSPOSED (QT/KT [e, s]),
    V in natural [s, e] layout - exactly the layouts the attention matmuls need.
  * Scores are computed transposed: ST[k, q] = sum_e KT[e,k] * QT[e,q]. With k on
    partitions, the post-softmax matrix is directly the stationary operand of the
    att @ V matmul (no on-chip transposes of the 2048x2048 matrix).
  * Softmax over k (= partition dim) needs only a SUM (no max subtraction: scores
    are bounded by ~ +-3 after scaling, exp is safe in fp32). The sum is a
    ones-vector matmul on the PE. Dropout keep-mask multiply is fused with the
    PSUM->SBUF evacuation of the scores; the 1/sqrt(768)/0.8 scale is folded into
    the ACT exp instruction's free affine scale.
  * Post-softmax mask: att = where(mask==0, 1e-9, att) is applied as att*mask
    in place (the 1e-9 branch is dropped: it perturbs the output by ~1.5e-6
    relative, far below fp16 operand rounding).
  * Normalization by 1/sum is deferred to the [q,e] output tile (768 wide)
    instead of the [k,q] attention matrix (2048 wide).
  * The attention stage is software-pipelined over q-chunks: the att@V matmuls
    of chunk i-1 are emitted between the score matmuls and the sum matmuls of
    chunk i, so the PE stays busy (and HAM stays at full clock) while chunk i's
    softmax runs on ACT/DVE.

Q/K biases ride the ACT PSUM->SBUF evacuation (per-partition bias with
func=Identity); the V bias is a K=1 ones-row matmul folded into its
accumulation group.
"""

import numpy as np

# ---------------------------------------------------------------------------
# Problem constants (hardcoded per harness contract)
# ---------------------------------------------------------------------------
B = 8
S = 2048
D = 768          # DIN == DOUT
DROP_P = 0.2
N_CORES = 8
P = 128          # partitions

_nc_cache = {}
_keep_cache = {}

QC = 512         # attention q-chunk width
CH = 512         # projection s-chunk width

# mask (post-softmax) stays fp16 so its DVE multiply hits the 2x perf mode;
# keep (pre-softmax, needed early) is u8 to halve its share of the startup DMA
MASK_DT_NP = np.float16
KEEP_DT_NP = np.uint8


def _build_nc(s=S, d=D, qc=QC, ch=CH):
    """Build + compile the per-core Bass program. Returns nc.

    s: sequence length, d: model dim, qc: q-chunk width for the attention
    stage, ch: s-chunk width for the projection stage.
    """
    import concourse.bacc as bacc
    import concourse.tile as tile
    from concourse import mybir

    DT = mybir.dt.float16
    F32 = mybir.dt.float32
    MDT = mybir.dt.from_np(MASK_DT_NP)
    KDT = mybir.dt.from_np(KEEP_DT_NP)

    dc = d // P            # contraction chunks for projections
    eb = d // P            # e-blocks of QT/KT
    nsc = s // ch          # projection s-chunks
    kb = s // P            # k-blocks
    nqc = s // qc          # attention q-chunks
    qs = qc // P           # q-subblocks per q-chunk
    eparts = []
    e0 = 0
    while e0 < d:
        eparts.append((e0, min(512, d - e0)))
        e0 += 512
    scale = 1.0 / (np.sqrt(float(d)) * (1.0 - DROP_P))

    nc = bacc.Bacc("TRN2", target_bir_lowering=False, debug=False,
                   num_devices=N_CORES)

    nsc_ = s // ch
    nqc_ = s // qc
    # activations: [P, nsc, dc, ch]; masks: [P, nqc, kb, qc]; W: [P, dc, d]
    qT = nc.dram_tensor("qT", [P, nsc_, d // P, ch], DT, kind="ExternalInput")
    kT = nc.dram_tensor("kT", [P, nsc_, d // P, ch], DT, kind="ExternalInput")
    vT = nc.dram_tensor("vT", [P, nsc_, d // P, ch], DT, kind="ExternalInput")
    maskT = nc.dram_tensor("maskT", [P, nqc_, s // P, qc], MDT,
                           kind="ExternalInput")
    keepT = nc.dram_tensor("keepT", [P, nqc_, s // P, qc], KDT,
                           kind="ExternalInput")
    Wq = nc.dram_tensor("Wq", [P, d // P, d], DT, kind="ExternalInput")
    Wk = nc.dram_tensor("Wk", [P, d // P, d], DT, kind="ExternalInput")
    Wv = nc.dram_tensor("Wv", [P, d // P, d], DT, kind="ExternalInput")
    # biases for Q/K arrive pre-chunked [P, d/P] (partition-major), V as [1, d]
    bq = nc.dram_tensor("bq", [P, d // P], F32, kind="ExternalInput")
    bk = nc.dram_tensor("bk", [P, d // P], F32, kind="ExternalInput")
    bv = nc.dram_tensor("bv", [1, d], DT, kind="ExternalInput")
    out = nc.dram_tensor("out", [s, d], F32, kind="ExternalOutput")

    with tile.TileContext(nc) as tc:
        with (
            tc.tile_pool(name="consts", bufs=1) as consts,
            tc.tile_pool(name="wpool", bufs=2) as wpool,
            tc.tile_pool(name="bigq", bufs=1) as bigq,
            tc.tile_pool(name="bigk", bufs=1) as bigk,
            tc.tile_pool(name="bigv", bufs=1) as bigv,
            tc.tile_pool(name="raw", bufs=3) as raw,
            tc.tile_pool(name="ppool", bufs=2) as ppool,
            tc.tile_pool(name="mpool", bufs=2) as mpool,
            tc.tile_pool(name="opool", bufs=3) as opool,
            tc.tile_pool(name="small", bufs=3) as small,
            tc.tile_pool(name="ps_st", bufs=3, space="PSUM") as ps_st,
            tc.tile_pool(name="ps_sum", bufs=1, space="PSUM") as ps_sum,
            tc.tile_pool(name="ps_o", bufs=2, space="PSUM") as ps_o,
        ):
            # constants
            ones_row = consts.tile([1, max(ch, 128)], DT, tag="ones_row")
            nc.vector.memset(ones_row, 1.0)
            ones_col = consts.tile([P, 1], DT, tag="ones_col")
            nc.vector.memset(ones_col, 1.0)
            one_f32 = consts.tile([1, 1], F32, tag="one_f32")
            nc.vector.memset(one_f32, 1.0)

            # tiles only - the bias DMAs are emitted after the first
            # weight/activation loads so they don't head-block the sync queue
            bq_sb = consts.tile([P, d // P], F32, tag="bq")
            bk_sb = consts.tile([P, d // P], F32, tag="bk")
            bv_sb = consts.tile([1, d], DT, tag="bv")

            def emit_bias_dmas():
                nc.sync.dma_start(out=bk_sb, in_=bk[:, :])
                nc.sync.dma_start(out=bq_sb, in_=bq[:, :])
                nc.sync.dma_start(out=bv_sb, in_=bv[:, :])

            QT_sb = bigq.tile([P, eb, s], DT, tag="QT")
            KT_sb = bigk.tile([P, eb, s], DT, tag="KT")
            V_sb = bigv.tile([P, kb, d], DT, tag="V")

            # PE warm-up: ~30 throwaway matmuls on resident constants while
            # the first weight/activation DMAs are in flight. Keeps the HAM
            # activity window busy so the real projections start at 2.4 GHz.
            warm_src = consts.tile([P, ch], DT, tag="warm_src")
            nc.vector.memset(warm_src, 0.0)
            warm_ps = ps_sum.tile([1, ch], F32, tag="den")
            for _ in range(18):
                nc.tensor.matmul(
                    warm_ps, lhsT=ones_col[:, 0:1], rhs=warm_src[:, :],
                    start=True, stop=True)

            # ---------------- Stage A: projections ----------------
            def emit_qk_proj(name, xdram, wdram):
                w_sb = wpool.tile([P, dc, d], DT, tag="W")
                if name == "k":
                    hc = dc // 2
                    nc.sync.dma_start(out=w_sb[:, 0:hc, :],
                                      in_=wdram[:, 0:hc, :])
                    nc.sync.dma_start(out=w_sb[:, hc:dc, :],
                                      in_=wdram[:, hc:dc, :])
                else:
                    nc.sync.dma_start(out=w_sb, in_=wdram[:, :, :])
                bias_sb = bk_sb if name == "k" else bq_sb
                dest = KT_sb if name == "k" else QT_sb
                for sc in range(nsc):
                    raw_sl = raw.tile([P, dc, ch], DT, tag="raw")
                    nc.sync.dma_start(out=raw_sl, in_=xdram[:, sc, :, :])
                    if name == "k" and sc == 0:
                        emit_bias_dmas()
                    for e in range(eb):
                        ps = ps_st.tile([P, ch], F32, tag="st")
                        for c in range(dc):
                            nc.tensor.matmul(
                                ps,
                                lhsT=w_sb[:, c, e * P:(e + 1) * P],
                                rhs=raw_sl[:, c, :],
                                start=(c == 0), stop=(c == dc - 1))
                        # evacuate + per-partition bias add
                        nc.scalar.activation(
                            out=dest[:, e, sc * ch:(sc + 1) * ch], in_=ps,
                            func=mybir.ActivationFunctionType.Identity,
                            bias=bias_sb[:, e:e + 1], scale=1.0)

            def emit_v_proj():
                w_sb = wpool.tile([P, dc, d], DT, tag="W")
                nc.sync.dma_start(out=w_sb, in_=Wv[:, :, :])
                for sc in range(nsc):
                    raw_sl = raw.tile([P, dc, ch], DT, tag="raw")
                    nc.sync.dma_start(out=raw_sl, in_=vT[:, sc, :, :])
                    for sbk in range(ch // P):
                        sb_g = sc * (ch // P) + sbk
                        ps = ps_o.tile([P, d], F32, tag="o")
                        for (e0, ew) in eparts:
                            for c in range(dc):
                                nc.tensor.matmul(
                                    ps[:, e0:e0 + ew],
                                    lhsT=raw_sl[:, c, sbk * P:(sbk + 1) * P],
                                    rhs=w_sb[:, c, e0:e0 + ew],
                                    start=(c == 0), stop=False)
                            nc.tensor.matmul(
                                ps[:, e0:e0 + ew],
                                lhsT=ones_row[0:1, 0:P],
                                rhs=bv_sb[0:1, e0:e0 + ew],
                                start=False, stop=True)
                        nc.scalar.copy(out=V_sb[:, sb_g, :], in_=ps)

            # ---------------- Stage B: attention (pipelined over q-chunks) --

            state = {}  # per-q-chunk tiles carried into the next iteration
            keep_tiles = {}

            def prefetch_keep(q0):
                h = kb // 2
                keep_sl = mpool.tile([P, kb, qc], KDT, tag="keep")
                nc.sync.dma_start(
                    out=keep_sl[:, 0:h, :], in_=keepT[:, q0, 0:h, :])
                nc.sync.dma_start(
                    out=keep_sl[:, h:kb, :], in_=keepT[:, q0, h:kb, :])
                keep_tiles[q0] = keep_sl

            def emit_scores(q0):
                qsl = slice(q0 * qc, (q0 + 1) * qc)
                if q0 not in keep_tiles:
                    prefetch_keep(q0)
                keep_sl = keep_tiles.pop(q0)
                P_sb = ppool.tile([P, kb, qc], DT, tag="P")
                for b in range(kb):
                    ps = ps_st.tile([P, qc], F32, tag="st")
                    for e in range(eb):
                        nc.tensor.matmul(
                            ps,
                            lhsT=KT_sb[:, e, b * P:(b + 1) * P],
                            rhs=QT_sb[:, e, qsl],
                            start=(e == 0), stop=(e == eb - 1))
                    nc.vector.tensor_mul(
                        out=P_sb[:, b, :], in0=ps, in1=keep_sl[:, b, :])
                state[q0] = (P_sb,)

            def emit_softmax(q0):
                (P_sb,) = state[q0]
                qsl = slice(q0 * qc, (q0 + 1) * qc)
                h = kb // 2
                mask_sl = mpool.tile([P, kb, qc], MDT, tag="mask")
                nc.sync.dma_start(
                    out=mask_sl[:, 0:h, :], in_=maskT[:, q0, 0:h, :])
                nc.sync.dma_start(
                    out=mask_sl[:, h:kb, :], in_=maskT[:, q0, h:kb, :])
                # exp in two halves so the sum matmuls can start earlier;
                # mask halves interleave right behind their sum reads
                nc.scalar.activation(
                    out=P_sb[:, 0:h, :], in_=P_sb[:, 0:h, :],
                    func=mybir.ActivationFunctionType.Exp, scale=scale)
                nc.scalar.activation(
                    out=P_sb[:, h:kb, :], in_=P_sb[:, h:kb, :],
                    func=mybir.ActivationFunctionType.Exp, scale=scale)
                sums = ps_sum.tile([1, qc], F32, tag="den")
                for b in range(kb):
                    nc.tensor.matmul(
                        sums, lhsT=ones_col[:, 0:1], rhs=P_sb[:, b, :],
                        start=(b == 0), stop=(b == kb - 1))
                    if b == h - 1:
                        nc.vector.tensor_mul(
                            out=P_sb[:, 0:h, :], in0=P_sb[:, 0:h, :],
                            in1=mask_sl[:, 0:h, :])
                nc.vector.tensor_mul(
                    out=P_sb[:, h:kb, :], in0=P_sb[:, h:kb, :],
                    in1=mask_sl[:, h:kb, :])
                sums_sb = small.tile([1, qc], F32, tag="sums_sb")
                nc.scalar.copy(out=sums_sb, in_=sums)
                den_sb = small.tile([P, qs], F32, tag="den_sb")
                for j in range(qs):
                    # shares the ps_sum slot (tag "den") with the sums tile
                    rt_ps = ps_sum.tile([P, 1], F32, tag="den")
                    nc.tensor.matmul(
                        rt_ps, lhsT=sums_sb[0:1, j * P:(j + 1) * P],
                        rhs=one_f32[0:1, 0:1], start=True, stop=True)
                    nc.scalar.copy(out=den_sb[:, j:j + 1], in_=rt_ps)
                rt_sb = small.tile([P, qs], F32, tag="rt_sb")
                nc.vector.reciprocal(out=rt_sb, in_=den_sb)
                state[q0] = (P_sb, rt_sb)

            def emit_pv(q0):
                P_sb, rt_sb = state.pop(q0)
                for j in range(qs):
                    o_ps = ps_o.tile([P, d], F32, tag="o")
                    for (e0, ew) in eparts:
                        for b in range(kb):
                            nc.tensor.matmul(
                                o_ps[:, e0:e0 + ew],
                                lhsT=P_sb[:, b, j * P:(j + 1) * P],
                                rhs=V_sb[:, b, e0:e0 + ew],
                                start=(b == 0), stop=(b == kb - 1))
                    o_sb = opool.tile([P, d], F32, tag="osb")
                    # evacuate + normalize in one ACT op (scale is per-partition)
                    nc.scalar.activation(
                        out=o_sb, in_=o_ps,
                        func=mybir.ActivationFunctionType.Identity,
                        scale=rt_sb[:, j:j + 1])
                    r0 = q0 * qc + j * P
                    nc.sync.dma_start(out=out[r0:r0 + P, :], in_=o_sb)

            emit_qk_proj("k", kT, Wk)
            prefetch_keep(0)
            emit_qk_proj("q", qT, Wq)
            emit_scores(0)
            emit_v_proj()       # PE work that overlaps softmax(0) on ACT/DVE
            emit_softmax(0)
            for q0 in range(1, nqc):
                emit_scores(q0)
                emit_pv(q0 - 1)
                emit_softmax(q0)
            emit_pv(nqc - 1)

    nc.compile()
    return nc


def _get_nc(key, **kw):
    if key not in _nc_cache:
        _nc_cache[key] = _build_nc(**kw)
    return _nc_cache[key]


def _keep_mask():
    """Bit-exact reproduction of the reference dropout mask, on CPU."""
    if "keep" not in _keep_cache:
        import jax

        cpu = jax.devices("cpu")[0]
        with jax.default_device(cpu):
            k = jax.random.bernoulli(
                jax.random.key(42), 1.0 - DROP_P, (B, S, S))
            # store pre-tiled per batch in the device dtype
            _keep_cache["keep"] = [
                _tile_pt(np.asarray(k[i]).T.astype(KEEP_DT_NP), S // QC, QC)
                for i in range(B)
            ]
    return _keep_cache["keep"]


def _chunk_bias(b):
    """[d] -> [P, d/P] partition-major chunking: out[p, e] = b[e*P + p]."""
    return np.ascontiguousarray(
        np.asarray(b, np.float32).reshape(D // P, P).T)


def _tile_pt(x2d, n_outer, w_outer):
    """[R, C] -> [P, n_outer, R//P, w_outer] with out[p, o, r, w] =
    x2d[r*P + p, o*w_outer + w]. Each [p, o] slice is one contiguous run,
    so per-chunk DMAs issue long descriptors instead of 512B fragments."""
    R, C = x2d.shape
    assert C == n_outer * w_outer
    return np.ascontiguousarray(
        x2d.reshape(R // P, P, n_outer, w_outer).transpose(1, 2, 0, 3))


def _prepare_in_maps(query, key, value, mask, Wq, bq, Wk, bk, Wv, bv):
    keep = _keep_mask()

    f16 = np.float16
    Wq16, Wk16, Wv16 = (
        _tile_pt(np.asarray(w, f16), 1, D)[:, 0] for w in (Wq, Wk, Wv))
    bq_c, bk_c = _chunk_bias(bq), _chunk_bias(bk)
    bv16 = np.asarray(bv, f16).reshape(1, D)

    in_maps = []
    for i in range(N_CORES):
        in_maps.append({
            "qT": _tile_pt(np.asarray(query[i]).T.astype(f16), S // CH, CH),
            "kT": _tile_pt(np.asarray(key[i]).T.astype(f16), S // CH, CH),
            "vT": _tile_pt(np.asarray(value[i]).T.astype(f16), S // CH, CH),
            "maskT": _tile_pt(
                np.asarray(mask[i]).T.astype(MASK_DT_NP), S // QC, QC),
            "keepT": keep[i],
            "Wq": Wq16, "Wk": Wk16, "Wv": Wv16,
            "bq": bq_c, "bk": bk_c, "bv": bv16,
        })
    return in_maps


def _install_ntff_hook():
    """Best-effort: provide antenv.axon_hooks (absent on this image) so
    run_bass_kernel_spmd can NTFF-profile if tracing is requested."""
    import sys
    import types

    if "antenv.axon_hooks" in sys.modules:
        return
    try:
        sys.path.insert(0, "/root/.axon_site")
        from trn_agent_boot.trn_boot import _ntff_profile_via_ctypes
        import antenv

        hook = _ntff_profile_via_ctypes("/opt/axon/libaxon_pjrt.so")
        mod = types.ModuleType("antenv.axon_hooks")
        mod._hook = hook
        mod.get_axon_ntff_profile_hook = lambda: mod._hook
        mod.set_axon_ntff_profile_hook = (
            lambda h: setattr(mod, "_hook", h))
        sys.modules["antenv.axon_hooks"] = mod
        antenv.axon_hooks = mod
    except Exception:
        pass


def kernel(query, key, value, mask, Wq, bq, Wk, bk, Wv, bv):
    from concourse.bass_utils import run_bass_kernel_spmd

    _install_ntff_hook()
    nc = _get_nc("full")
    in_maps = _prepare_in_maps(
        query, key, value, mask, Wq, bq, Wk, bk, Wv, bv)
    res = run_bass_kernel_spmd(nc, in_maps, list(range(N_CORES)))
    return np.stack([res.results[i]["out"] for i in range(N_CORES)], axis=0)


# revision 34
# speedup vs baseline: 1.0079x; 1.0079x over previous
"""Trainium2 Bass kernel for nn_DotProductAttention_61529701482813.

Reference computation (per batch b):
    Q = query @ Wq + bq ; K = key @ Wk + bk ; V = value @ Wv + bv
    S = Q @ K^T / sqrt(768)
    S = where(keep, S / 0.8, 0)        # dropout on raw scores, keep ~ Bern(0.8), jax key(42)
    att = softmax(S, axis=-1)
    att = where(mask == 0, 1e-9, att)  # mask applied AFTER softmax
    out = att @ V

Sharding: pure data parallel, batch b -> core b (B == n_cores == 8). No collectives.

Per-core kernel layout strategy:
  * All matmul operands are fp16 (PE runs 1 cycle/row for fp16 vs 4 for fp32);
    accumulation is always fp32 in PSUM.
  * Projections consume host-transposed activations xT [D, S] so the contraction
    dim (d) lands on partitions. Q and K are produced TRANSPOSED (QT/KT [e, s]),
    V in natural [s, e] layout - exactly the layouts the attention matmuls need.
  * Scores are computed transposed: ST[k, q] = sum_e KT[e,k] * QT[e,q]. With k on
    partitions, the post-softmax matrix is directly the stationary operand of the
    att @ V matmul (no on-chip transposes of the 2048x2048 matrix).
  * Softmax over k (= partition dim) needs only a SUM (no max subtraction: scores
    are bounded by ~ +-3 after scaling, exp is safe in fp32). The sum is a
    ones-vector matmul on the PE. Dropout keep-mask multiply is fused with the
    PSUM->SBUF evacuation of the scores; the 1/sqrt(768)/0.8 scale is folded into
    the ACT exp instruction's free affine scale.
  * Post-softmax mask: att = where(mask==0, 1e-9, att) is applied as att*mask
    in place (the 1e-9 branch is dropped: it perturbs the output by ~1.5e-6
    relative, far below fp16 operand rounding).
  * Normalization by 1/sum is deferred to the [q,e] output tile (768 wide)
    instead of the [k,q] attention matrix (2048 wide).
  * The attention stage is software-pipelined over q-chunks: the att@V matmuls
    of chunk i-1 are emitted between the score matmuls and the sum matmuls of
    chunk i, so the PE stays busy (and HAM stays at full clock) while chunk i's
    softmax runs on ACT/DVE.

Q/K biases ride the ACT PSUM->SBUF evacuation (per-partition bias with
func=Identity); the V bias is a K=1 ones-row matmul folded into its
accumulation group.
"""

import numpy as np

# ---------------------------------------------------------------------------
# Problem constants (hardcoded per harness contract)
# ---------------------------------------------------------------------------
B = 8
S = 2048
D = 768          # DIN == DOUT
DROP_P = 0.2
N_CORES = 8
P = 128          # partitions

_nc_cache = {}
_keep_cache = {}

QC = 512         # attention q-chunk width
CH = 512         # projection s-chunk width

# mask (post-softmax) stays fp16 so its DVE multiply hits the 2x perf mode;
# keep (pre-softmax, needed early) is u8 to halve its share of the startup DMA
MASK_DT_NP = np.float16
KEEP_DT_NP = np.uint8


def _build_nc(s=S, d=D, qc=QC, ch=CH):
    """Build + compile the per-core Bass program. Returns nc.

    s: sequence length, d: model dim, qc: q-chunk width for the attention
    stage, ch: s-chunk width for the projection stage.
    """
    import concourse.bacc as bacc
    import concourse.tile as tile
    from concourse import mybir

    DT = mybir.dt.float16
    F32 = mybir.dt.float32
    MDT = mybir.dt.from_np(MASK_DT_NP)
    KDT = mybir.dt.from_np(KEEP_DT_NP)

    dc = d // P            # contraction chunks for projections
    eb = d // P            # e-blocks of QT/KT
    nsc = s // ch          # projection s-chunks
    kb = s // P            # k-blocks
    nqc = s // qc          # attention q-chunks
    qs = qc // P           # q-subblocks per q-chunk
    eparts = []
    e0 = 0
    while e0 < d:
        eparts.append((e0, min(512, d - e0)))
        e0 += 512
    scale = 1.0 / (np.sqrt(float(d)) * (1.0 - DROP_P))

    nc = bacc.Bacc("TRN2", target_bir_lowering=False, debug=False,
                   num_devices=N_CORES)

    nsc_ = s // ch
    nqc_ = s // qc
    # activations: [P, nsc, dc, ch]; masks: [P, nqc, kb, qc]; W: [P, dc, d]
    qT = nc.dram_tensor("qT", [P, nsc_, d // P, ch], DT, kind="ExternalInput")
    kT = nc.dram_tensor("kT", [P, nsc_, d // P, ch], DT, kind="ExternalInput")
    vT = nc.dram_tensor("vT", [P, nsc_, d // P, ch], DT, kind="ExternalInput")
    maskT = nc.dram_tensor("maskT", [P, nqc_, s // P, qc], MDT,
                           kind="ExternalInput")
    keepT = nc.dram_tensor("keepT", [P, nqc_, s // P, qc], KDT,
                           kind="ExternalInput")
    Wq = nc.dram_tensor("Wq", [P, d // P, d], DT, kind="ExternalInput")
    Wk = nc.dram_tensor("Wk", [P, d // P, d], DT, kind="ExternalInput")
    Wv = nc.dram_tensor("Wv", [P, d // P, d], DT, kind="ExternalInput")
    # biases for Q/K arrive pre-chunked [P, d/P] (partition-major), V as [1, d]
    bq = nc.dram_tensor("bq", [P, d // P], F32, kind="ExternalInput")
    bk = nc.dram_tensor("bk", [P, d // P], F32, kind="ExternalInput")
    bv = nc.dram_tensor("bv", [1, d], DT, kind="ExternalInput")
    out = nc.dram_tensor("out", [s, d], F32, kind="ExternalOutput")

    with tile.TileContext(nc) as tc:
        with (
            tc.tile_pool(name="consts", bufs=1) as consts,
            tc.tile_pool(name="wpool", bufs=2) as wpool,
            tc.tile_pool(name="bigq", bufs=1) as bigq,
            tc.tile_pool(name="bigk", bufs=1) as bigk,
            tc.tile_pool(name="bigv", bufs=1) as bigv,
            tc.tile_pool(name="raw", bufs=3) as raw,
            tc.tile_pool(name="ppool", bufs=2) as ppool,
            tc.tile_pool(name="mpool", bufs=2) as mpool,
            tc.tile_pool(name="opool", bufs=3) as opool,
            tc.tile_pool(name="small", bufs=3) as small,
            tc.tile_pool(name="ps_st", bufs=3, space="PSUM") as ps_st,
            tc.tile_pool(name="ps_sum", bufs=1, space="PSUM") as ps_sum,
            tc.tile_pool(name="ps_o", bufs=2, space="PSUM") as ps_o,
        ):
            # constants
            ones_row = consts.tile([1, max(ch, 128)], DT, tag="ones_row")
            nc.vector.memset(ones_row, 1.0)
            ones_col = consts.tile([P, 1], DT, tag="ones_col")
            nc.vector.memset(ones_col, 1.0)
            one_f32 = consts.tile([1, 1], F32, tag="one_f32")
            nc.vector.memset(one_f32, 1.0)

            # tiles only - the bias DMAs are emitted after the first
            # weight/activation loads so they don't head-block the sync queue
            bq_sb = consts.tile([P, d // P], F32, tag="bq")
            bk_sb = consts.tile([P, d // P], F32, tag="bk")
            bv_sb = consts.tile([1, d], DT, tag="bv")

            def emit_bias_dmas():
                nc.sync.dma_start(out=bk_sb, in_=bk[:, :])
                nc.sync.dma_start(out=bq_sb, in_=bq[:, :])
                nc.sync.dma_start(out=bv_sb, in_=bv[:, :])

            QT_sb = bigq.tile([P, eb, s], DT, tag="QT")
            KT_sb = bigk.tile([P, eb, s], DT, tag="KT")
            V_sb = bigv.tile([P, kb, d], DT, tag="V")

            # PE warm-up: ~30 throwaway matmuls on resident constants while
            # the first weight/activation DMAs are in flight. Keeps the HAM
            # activity window busy so the real projections start at 2.4 GHz.
            warm_src = consts.tile([P, ch], DT, tag="warm_src")
            nc.vector.memset(warm_src, 0.0)
            warm_ps = ps_sum.tile([1, ch], F32, tag="den")
            for _ in range(30):
                nc.tensor.matmul(
                    warm_ps, lhsT=ones_col[:, 0:1], rhs=warm_src[:, :],
                    start=True, stop=True)

            # ---------------- Stage A: projections ----------------
            def emit_qk_proj(name, xdram, wdram):
                w_sb = wpool.tile([P, dc, d], DT, tag="W")
                if name == "k":
                    hc = dc // 2
                    nc.sync.dma_start(out=w_sb[:, 0:hc, :],
                                      in_=wdram[:, 0:hc, :])
                    nc.sync.dma_start(out=w_sb[:, hc:dc, :],
                                      in_=wdram[:, hc:dc, :])
                else:
                    nc.sync.dma_start(out=w_sb, in_=wdram[:, :, :])
                bias_sb = bk_sb if name == "k" else bq_sb
                dest = KT_sb if name == "k" else QT_sb
                for sc in range(nsc):
                    raw_sl = raw.tile([P, dc, ch], DT, tag="raw")
                    nc.sync.dma_start(out=raw_sl, in_=xdram[:, sc, :, :])
                    if name == "k" and sc == 0:
                        emit_bias_dmas()
                    for e in range(eb):
                        ps = ps_st.tile([P, ch], F32, tag="st")
                        for c in range(dc):
                            nc.tensor.matmul(
                                ps,
                                lhsT=w_sb[:, c, e * P:(e + 1) * P],
                                rhs=raw_sl[:, c, :],
                                start=(c == 0), stop=(c == dc - 1))
                        # evacuate + per-partition bias add
                        nc.scalar.activation(
                            out=dest[:, e, sc * ch:(sc + 1) * ch], in_=ps,
                            func=mybir.ActivationFunctionType.Identity,
                            bias=bias_sb[:, e:e + 1], scale=1.0)

            def emit_v_proj():
                w_sb = wpool.tile([P, dc, d], DT, tag="W")
                nc.sync.dma_start(out=w_sb, in_=Wv[:, :, :])
                for sc in range(nsc):
                    raw_sl = raw.tile([P, dc, ch], DT, tag="raw")
                    nc.sync.dma_start(out=raw_sl, in_=vT[:, sc, :, :])
                    for sbk in range(ch // P):
                        sb_g = sc * (ch // P) + sbk
                        ps = ps_o.tile([P, d], F32, tag="o")
                        for (e0, ew) in eparts:
                            for c in range(dc):
                                nc.tensor.matmul(
                                    ps[:, e0:e0 + ew],
                                    lhsT=raw_sl[:, c, sbk * P:(sbk + 1) * P],
                                    rhs=w_sb[:, c, e0:e0 + ew],
                                    start=(c == 0), stop=False)
                            nc.tensor.matmul(
                                ps[:, e0:e0 + ew],
                                lhsT=ones_row[0:1, 0:P],
                                rhs=bv_sb[0:1, e0:e0 + ew],
                                start=False, stop=True)
                        nc.scalar.copy(out=V_sb[:, sb_g, :], in_=ps)

            # ---------------- Stage B: attention (pipelined over q-chunks) --

            state = {}  # per-q-chunk tiles carried into the next iteration
            keep_tiles = {}

            def prefetch_keep(q0):
                h = kb // 2
                keep_sl = mpool.tile([P, kb, qc], KDT, tag="keep")
                nc.sync.dma_start(
                    out=keep_sl[:, 0:h, :], in_=keepT[:, q0, 0:h, :])
                nc.sync.dma_start(
                    out=keep_sl[:, h:kb, :], in_=keepT[:, q0, h:kb, :])
                keep_tiles[q0] = keep_sl

            def emit_scores(q0):
                qsl = slice(q0 * qc, (q0 + 1) * qc)
                if q0 not in keep_tiles:
                    prefetch_keep(q0)
                keep_sl = keep_tiles.pop(q0)
                P_sb = ppool.tile([P, kb, qc], DT, tag="P")
                for b in range(kb):
                    ps = ps_st.tile([P, qc], F32, tag="st")
                    for e in range(eb):
                        nc.tensor.matmul(
                            ps,
                            lhsT=KT_sb[:, e, b * P:(b + 1) * P],
                            rhs=QT_sb[:, e, qsl],
                            start=(e == 0), stop=(e == eb - 1))
                    nc.vector.tensor_mul(
                        out=P_sb[:, b, :], in0=ps, in1=keep_sl[:, b, :])
                state[q0] = (P_sb,)

            def emit_softmax(q0):
                (P_sb,) = state[q0]
                qsl = slice(q0 * qc, (q0 + 1) * qc)
                h = kb // 2
                mask_sl = mpool.tile([P, kb, qc], MDT, tag="mask")
                nc.sync.dma_start(
                    out=mask_sl[:, 0:h, :], in_=maskT[:, q0, 0:h, :])
                nc.sync.dma_start(
                    out=mask_sl[:, h:kb, :], in_=maskT[:, q0, h:kb, :])
                # exp in two halves so the sum matmuls can start earlier;
                # mask halves interleave right behind their sum reads
                nc.scalar.activation(
                    out=P_sb[:, 0:h, :], in_=P_sb[:, 0:h, :],
                    func=mybir.ActivationFunctionType.Exp, scale=scale)
                nc.scalar.activation(
                    out=P_sb[:, h:kb, :], in_=P_sb[:, h:kb, :],
                    func=mybir.ActivationFunctionType.Exp, scale=scale)
                sums = ps_sum.tile([1, qc], F32, tag="den")
                for b in range(kb):
                    nc.tensor.matmul(
                        sums, lhsT=ones_col[:, 0:1], rhs=P_sb[:, b, :],
                        start=(b == 0), stop=(b == kb - 1))
                    if b == h - 1:
                        nc.vector.tensor_mul(
                            out=P_sb[:, 0:h, :], in0=P_sb[:, 0:h, :],
                            in1=mask_sl[:, 0:h, :])
                nc.vector.tensor_mul(
                    out=P_sb[:, h:kb, :], in0=P_sb[:, h:kb, :],
                    in1=mask_sl[:, h:kb, :])
                sums_sb = small.tile([1, qc], F32, tag="sums_sb")
                nc.vector.tensor_copy(out=sums_sb, in_=sums)
                den_sb = small.tile([P, qs], F32, tag="den_sb")
                for j in range(qs):
                    # shares the ps_sum slot (tag "den") with the sums tile
                    rt_ps = ps_sum.tile([P, 1], F32, tag="den")
                    nc.tensor.matmul(
                        rt_ps, lhsT=sums_sb[0:1, j * P:(j + 1) * P],
                        rhs=one_f32[0:1, 0:1], start=True, stop=True)
                    nc.vector.tensor_copy(out=den_sb[:, j:j + 1], in_=rt_ps)
                rt_sb = small.tile([P, qs], F32, tag="rt_sb")
                nc.vector.reciprocal(out=rt_sb, in_=den_sb)
                state[q0] = (P_sb, rt_sb)

            def emit_pv(q0):
                P_sb, rt_sb = state.pop(q0)
                for j in range(qs):
                    o_ps = ps_o.tile([P, d], F32, tag="o")
                    for (e0, ew) in eparts:
                        for b in range(kb):
                            nc.tensor.matmul(
                                o_ps[:, e0:e0 + ew],
                                lhsT=P_sb[:, b, j * P:(j + 1) * P],
                                rhs=V_sb[:, b, e0:e0 + ew],
                                start=(b == 0), stop=(b == kb - 1))
                    o_sb = opool.tile([P, d], F32, tag="osb")
                    # evacuate + normalize in one ACT op (scale is per-partition)
                    nc.scalar.activation(
                        out=o_sb, in_=o_ps,
                        func=mybir.ActivationFunctionType.Identity,
                        scale=rt_sb[:, j:j + 1])
                    r0 = q0 * qc + j * P
                    nc.sync.dma_start(out=out[r0:r0 + P, :], in_=o_sb)

            emit_qk_proj("k", kT, Wk)
            prefetch_keep(0)
            emit_qk_proj("q", qT, Wq)
            emit_scores(0)
            emit_v_proj()       # PE work that overlaps softmax(0) on ACT/DVE
            emit_softmax(0)
            for q0 in range(1, nqc):
                emit_scores(q0)
                emit_pv(q0 - 1)
                emit_softmax(q0)
            emit_pv(nqc - 1)

    nc.compile()
    return nc


def _get_nc(key, **kw):
    if key not in _nc_cache:
        _nc_cache[key] = _build_nc(**kw)
    return _nc_cache[key]


def _keep_mask():
    """Bit-exact reproduction of the reference dropout mask, on CPU."""
    if "keep" not in _keep_cache:
        import jax

        cpu = jax.devices("cpu")[0]
        with jax.default_device(cpu):
            k = jax.random.bernoulli(
                jax.random.key(42), 1.0 - DROP_P, (B, S, S))
            # store pre-tiled per batch in the device dtype
            _keep_cache["keep"] = [
                _tile_pt(np.asarray(k[i]).T.astype(KEEP_DT_NP), S // QC, QC)
                for i in range(B)
            ]
    return _keep_cache["keep"]


def _chunk_bias(b):
    """[d] -> [P, d/P] partition-major chunking: out[p, e] = b[e*P + p]."""
    return np.ascontiguousarray(
        np.asarray(b, np.float32).reshape(D // P, P).T)


def _tile_pt(x2d, n_outer, w_outer):
    """[R, C] -> [P, n_outer, R//P, w_outer] with out[p, o, r, w] =
    x2d[r*P + p, o*w_outer + w]. Each [p, o] slice is one contiguous run,
    so per-chunk DMAs issue long descriptors instead of 512B fragments."""
    R, C = x2d.shape
    assert C == n_outer * w_outer
    return np.ascontiguousarray(
        x2d.reshape(R // P, P, n_outer, w_outer).transpose(1, 2, 0, 3))


def _prepare_in_maps(query, key, value, mask, Wq, bq, Wk, bk, Wv, bv):
    keep = _keep_mask()

    f16 = np.float16
    Wq16, Wk16, Wv16 = (
        _tile_pt(np.asarray(w, f16), 1, D)[:, 0] for w in (Wq, Wk, Wv))
    bq_c, bk_c = _chunk_bias(bq), _chunk_bias(bk)
    bv16 = np.asarray(bv, f16).reshape(1, D)

    in_maps = []
    for i in range(N_CORES):
        in_maps.append({
            "qT": _tile_pt(np.asarray(query[i]).T.astype(f16), S // CH, CH),
            "kT": _tile_pt(np.asarray(key[i]).T.astype(f16), S // CH, CH),
            "vT": _tile_pt(np.asarray(value[i]).T.astype(f16), S // CH, CH),
            "maskT": _tile_pt(
                np.asarray(mask[i]).T.astype(MASK_DT_NP), S // QC, QC),
            "keepT": keep[i],
            "Wq": Wq16, "Wk": Wk16, "Wv": Wv16,
            "bq": bq_c, "bk": bk_c, "bv": bv16,
        })
    return in_maps


def _install_ntff_hook():
    """Best-effort: provide antenv.axon_hooks (absent on this image) so
    run_bass_kernel_spmd can NTFF-profile if tracing is requested."""
    import sys
    import types

    if "antenv.axon_hooks" in sys.modules:
        return
    try:
        sys.path.insert(0, "/root/.axon_site")
        from trn_agent_boot.trn_boot import _ntff_profile_via_ctypes
        import antenv

        hook = _ntff_profile_via_ctypes("/opt/axon/libaxon_pjrt.so")
        mod = types.ModuleType("antenv.axon_hooks")
        mod._hook = hook
        mod.get_axon_ntff_profile_hook = lambda: mod._hook
        mod.set_axon_ntff_profile_hook = (
            lambda h: setattr(mod, "_hook", h))
        sys.modules["antenv.axon_hooks"] = mod
        antenv.axon_hooks = mod
    except Exception:
        pass


def kernel(query, key, value, mask, Wq, bq, Wk, bk, Wv, bv):
    from concourse.bass_utils import run_bass_kernel_spmd

    _install_ntff_hook()
    nc = _get_nc("full")
    in_maps = _prepare_in_maps(
        query, key, value, mask, Wq, bq, Wk, bk, Wv, bv)
    res = run_bass_kernel_spmd(nc, in_maps, list(range(N_CORES)))
    return np.stack([res.results[i]["out"] for i in range(N_CORES)], axis=0)
